# revision 21
# baseline (speedup 1.0000x reference)
"""CapsuleLayer forward on 8 Trainium2 NeuronCores.

The reference collapses algebraically: routing runs exactly one iteration
with uniform coefficients c = 1/R, so

    s[b, (n,o)] = (1/R) * sum_{r,i} x[b,r,i] * W[n,r,i,o]
                = (1/R) * (x_flat @ W_flat)[b, (n,o)]
    v = squash(s) over o

i.e. one [256, 9216] @ [9216, 160] matmul plus a tiny squash on 40960
elements. u_hat ([10,256,1152,16], 189 MB) is never materialized.

Sharding: contraction-dim (K = R*CIN) sharding across the 8 cores — each
core reads only 1/8 of x^T and 1/8 of W (1.9 MB vs 7 MB/core for batch
sharding) and the PE does 4x fewer matmul instructions (full 128-wide
stationary operand). Each core emits its raw partial product (s^T layout,
[160, 256]); the host sums the 8 partials and applies the tiny squash
while unsharding. (An on-device ReduceScatter was measured at ~46 us —
a 32 us launch-skew barrier plus 14 us transfer — so cross-core
reduction on device is strictly worse.)

Matmuls run as float32r (fp32 bits, replicated PE mode): with a 256-wide
moving operand this runs at 1 cycle/row vs 4 for plain fp32.
"""

import numpy as np
from contextlib import ExitStack

import concourse.bass as bass
import concourse.tile as tile
from concourse import bacc, mybir
from concourse.bass_utils import run_bass_kernel_spmd

N_CAPS, R, CIN, COUT = 10, 1152, 8, 16
B = 256
NCORES = 8
K = R * CIN            # 9216 contraction length
KSH = K // NCORES      # 1152 contraction slice per core
NO = N_CAPS * COUT     # 160 output rows (s^T layout)
P = 128
KT = KSH // P          # 9 k-tiles per core

F32 = mybir.dt.float32
F32R = mybir.dt.float32r

_built = None


# Pipelined input chunks: chunk 0 covers k-tiles [0, KSPLIT), chunk 1 the
# rest. x^T is split into 64-aligned partition bands (one per HW queue);
# partition bands NOT aligned to 64 degenerate into 4-byte DMA
# descriptors (~15 GB/s — measured), so only 0:64/64:128 are used.
KSPLIT = 4


def _build_nc():
    nc = bacc.Bacc(
        "TRN2", target_bir_lowering=False, debug=False, num_devices=NCORES
    )
    xt = nc.dram_tensor("xt", [P, KT * B], F32R, kind="ExternalInput").ap()
    wk = nc.dram_tensor("wk", [P, KT * NO], F32R, kind="ExternalInput").ap()
    out = nc.dram_tensor("out", [NO, B], F32, kind="ExternalOutput").ap()

    with tile.TileContext(nc) as tc, ExitStack() as ctx:
        xp = ctx.enter_context(tc.tile_pool(name="xp", bufs=2))
        wp = ctx.enter_context(tc.tile_pool(name="wp", bufs=2))
        pp = ctx.enter_context(tc.tile_pool(name="pp", bufs=1, space="PSUM"))

        # PE warm-up: the HAM clock gate keeps the PE at 1.2 GHz until
        # it has seen ~3.4 us of sustained activity. Spend the DMA-fill
        # window running throwaway matmuls on scratch tiles so the real
        # matmuls run at 2.4 GHz.
        warm = ctx.enter_context(tc.tile_pool(name="warm", bufs=1))
        wsrc = warm.tile([P, P], F32, tag="wsrc")
        wps = pp.tile([P, P], F32, tag="wps")
        nc.gpsimd.memset(wsrc[:], 0.0)
        for _ in range(12):
            nc.tensor.matmul(wps[:], wsrc[:], wsrc[:], start=True, stop=True)

        # s^T partial: [160, 256] across two PSUM tiles (stationary W
        # tile is limited to 128 columns).
        psA = pp.tile([P, B], F32, tag="psA")
        psB = pp.tile([NO - P, B], F32, tag="psB")

        # Two pipelined chunks; within each chunk x^T rides two HW queues
        # (64-aligned partition bands) and W the third. Large contiguous
        # per-partition descriptors keep per-queue throughput up.
        xt_t = xp.tile([P, KT * B], F32R)
        wk_t = wp.tile([P, KT * NO], F32R)
        H = P // 2
        for k0, k1 in ((0, KSPLIT), (KSPLIT, KT)):
            nc.sync.dma_start(
                xt_t[0:H, k0 * B:k1 * B], xt[0:H, k0 * B:k1 * B]
            )
            nc.gpsimd.dma_start(
                xt_t[H:P, k0 * B:k1 * B], xt[H:P, k0 * B:k1 * B]
            )
            nc.scalar.dma_start(
                wk_t[:, k0 * NO:k1 * NO], wk[:, k0 * NO:k1 * NO]
            )

        # Matmuls trail chunk arrival; within each chunk psA's group runs
        # first so its PSUM eviction overlaps psB's remaining matmuls.
        order = (
            [(j, 0) for j in range(KSPLIT)]
            + [(j, 1) for j in range(KSPLIT)]
            + [(j, 0) for j in range(KSPLIT, KT)]
            + [(j, 1) for j in range(KSPLIT, KT)]
        )
        for j, h in order:
            if h == 0:
                nc.tensor.matmul(
                    psA[:],
                    wk_t[:, j * NO:j * NO + P],
                    xt_t[:, j * B:(j + 1) * B],
                    start=(j == 0), stop=(j == KT - 1),
                )
            else:
                nc.tensor.matmul(
                    psB[:],
                    wk_t[:, j * NO + P:(j + 1) * NO],
                    xt_t[:, j * B:(j + 1) * B],
                    start=(j == 0), stop=(j == KT - 1),
                )
        sb = ctx.enter_context(tc.tile_pool(name="sb", bufs=1))
        sA = sb.tile([P, B], F32, tag="sA")
        sB = sb.tile([NO - P, B], F32, tag="sB")
        nc.vector.tensor_copy(sA[:], psA[:])
        nc.vector.tensor_copy(sB[:], psB[:])
        nc.sync.dma_start(out[0:P // 2, :], sA[0:P // 2, :])
        nc.gpsimd.dma_start(out[P // 2:P, :], sA[P // 2:P, :])
        nc.scalar.dma_start(out[P:NO, :], sB[:])

    nc.compile()
    return nc


def _build_nc_raw():
    """Raw bacc (no TileContext): manual semaphores, no Tile preamble
    barrier / tail drain+butterfly. Same dataflow as _build_nc."""
    nc = bacc.Bacc(
        "TRN2", target_bir_lowering=False, debug=False, num_devices=NCORES
    )
    xt = nc.dram_tensor("xt", [P, KT * B], F32R, kind="ExternalInput").ap()
    wk = nc.dram_tensor("wk", [P, KT * NO], F32R, kind="ExternalInput").ap()
    out = nc.dram_tensor("out", [NO, B], F32, kind="ExternalOutput").ap()

    H = P // 2
    with ExitStack() as ctx:
        cleanup = ctx.enter_context(nc.cleanup_on_exit())  # noqa: F841
        xt_sb = ctx.enter_context(nc.sbuf_tensor("xts", [P, KT * B], F32R)).ap()
        wk_sb = ctx.enter_context(nc.sbuf_tensor("wks", [P, KT * NO], F32R)).ap()
        sA = ctx.enter_context(nc.sbuf_tensor("sA", [P, B], F32)).ap()
        sB = ctx.enter_context(nc.sbuf_tensor("sB", [NO - P, B], F32)).ap()
        wsrc = ctx.enter_context(nc.sbuf_tensor("wsrc", [P, P], F32)).ap()
        # full-bank PSUM allocations so psA/psB/warm never share a bank
        # (PE-write + DVE-read in one bank is fatal)
        psA = ctx.enter_context(nc.psum_tensor("psA", [P, 512], F32)).ap()
        psB = ctx.enter_context(nc.psum_tensor("psB", [P, 512], F32)).ap()
        wps = ctx.enter_context(nc.psum_tensor("wps", [P, 512], F32)).ap()

        block_ctx = ExitStack()
        block = block_ctx.enter_context(nc.Block())
        ws = nc.alloc_semaphore("ws")
        dxs = nc.alloc_semaphore("dxs")
        dxg = nc.alloc_semaphore("dxg")
        dws = nc.alloc_semaphore("dws")
        mmA = nc.alloc_semaphore("mmA")
        mmB = nc.alloc_semaphore("mmB")
        cpA = nc.alloc_semaphore("cpA")
        cpB = nc.alloc_semaphore("cpB")
        ods = nc.alloc_semaphore("ods")
        odg = nc.alloc_semaphore("odg")
        odc = nc.alloc_semaphore("odc")

        @block.sync
        def _(eng: bass.BassEngine):
            for i, (k0, k1) in enumerate(((0, KSPLIT), (KSPLIT, KT))):
                eng.dma_start(
                    out=xt_sb[0:H, k0 * B:k1 * B], in_=xt[0:H, k0 * B:k1 * B]
                ).then_inc(dxs, 16)
            eng.wait_ge(cpA, 1)
            eng.dma_start(out=out[0:H, :], in_=sA[0:H, :]).then_inc(ods, 16)
            eng.wait_ge(ods, 16)

        @block.gpsimd
        def _(eng: bass.BassEngine):
            eng.memset(wsrc[:], 0.0).then_inc(ws, 1)
            for k0, k1 in ((0, KSPLIT), (KSPLIT, KT)):
                eng.dma_start(
                    out=xt_sb[H:P, k0 * B:k1 * B], in_=xt[H:P, k0 * B:k1 * B]
                ).then_inc(dxg, 16)
            eng.wait_ge(cpA, 1)
            eng.dma_start(out=out[H:P, :], in_=sA[H:P, :]).then_inc(odg, 16)
            eng.wait_ge(odg, 16)

        @block.scalar
        def _(eng: bass.BassEngine):
            for k0, k1 in ((0, KSPLIT), (KSPLIT, KT)):
                eng.dma_start(
                    out=wk_sb[:, k0 * NO:k1 * NO], in_=wk[:, k0 * NO:k1 * NO]
                ).then_inc(dws, 16)
            eng.wait_ge(cpB, 1)
            eng.dma_start(out=out[P:NO, :], in_=sB[:, :]).then_inc(odc, 16)
            eng.wait_ge(odc, 16)

        @block.tensor
        def _(eng):
            eng.wait_ge(ws, 1)
            for _ in range(12):
                nc.tensor.matmul(
                    wps[:, 0:P], wsrc[:], wsrc[:], start=True, stop=True
                )
            for ci, (k0, k1) in enumerate(((0, KSPLIT), (KSPLIT, KT))):
                eng.wait_ge(dxs, 16 * (ci + 1))
                eng.wait_ge(dxg, 16 * (ci + 1))
                eng.wait_ge(dws, 16 * (ci + 1))
                for j in range(k0, k1):
                    i = nc.tensor.matmul(
                        psA[:, 0:B],
                        wk_sb[:, j * NO:j * NO + P],
                        xt_sb[:, j * B:(j + 1) * B],
                        start=(j == k0 and ci == 0), stop=(j == KT - 1),
                    )
                    if j == KT - 1:
                        i.then_inc(mmA, 1)
                for j in range(k0, k1):
                    i = nc.tensor.matmul(
                        psB[0:NO - P, 0:B],
                        wk_sb[:, j * NO + P:(j + 1) * NO],
                        xt_sb[:, j * B:(j + 1) * B],
                        start=(j == k0 and ci == 0), stop=(j == KT - 1),
                    )
                    if j == KT - 1:
                        i.then_inc(mmB, 1)

        @block.vector
        def _(eng):
            eng.wait_ge(mmA, 1)
            eng.tensor_copy(sA[:], psA[:, 0:B]).then_inc(cpA, 1)
            eng.wait_ge(mmB, 1)
            eng.tensor_copy(sB[:], psB[0:NO - P, 0:B]).then_inc(cpB, 1)

        block_ctx.close()
        # cleanup_on_exit requires the body to end at a barrier that has
        # retired every engine's pending semaphore updates.
        nc.all_engine_barrier()

    nc.compile()
    return nc


RAW = True


def _get_nc():
    global _built
    if _built is None:
        _built = _build_nc_raw() if RAW else _build_nc()
    return _built


def _make_in_maps(x, W):
    x = np.asarray(x, dtype=np.float32)
    W = np.asarray(W, dtype=np.float32)
    # x^T: [K, B]; W to [K, NO] with k = r*CIN + i matching x's flattening.
    # Then pack k-tile-major per core: [NCORES, P, KT * cols] where row p
    # holds k-tile k's p-th contraction row at column block k.
    xt_full = x.reshape(B, K).T  # [K, B] view
    wk_full = W.transpose(1, 2, 0, 3).reshape(K, NO)
    xt_pack = np.ascontiguousarray(
        xt_full.reshape(NCORES, KT, P, B).transpose(0, 2, 1, 3)
    ).reshape(NCORES, P, KT * B)
    wk_pack = np.ascontiguousarray(
        wk_full.reshape(NCORES, KT, P, NO).transpose(0, 2, 1, 3)
    ).reshape(NCORES, P, KT * NO)
    return [{"xt": xt_pack[c], "wk": wk_pack[c]} for c in range(NCORES)]


def _assemble(results):
    # Sum the 8 K-slice partials (the "all-reduce" leg of unsharding),
    # then apply squash: with t = raw sum (s = t/R, ssq = sum_o t^2),
    #   v = t * sqrt(ssq) / (R^2 + ssq)
    t = np.zeros((NO, B), dtype=np.float32)
    for c in range(NCORES):
        t += results[c]["out"]
    t = t.T.reshape(B, N_CAPS, COUT).astype(np.float64)
    ssq = np.sum(t * t, axis=-1, keepdims=True)
    v = t * np.sqrt(ssq) / (R * R + ssq)
    return np.ascontiguousarray(
        v.transpose(1, 0, 2)[:, :, None, None, :]
    ).astype(np.float32)


def _run(x, W, **spmd_kwargs):
    nc = _get_nc()
    in_maps = _make_in_maps(x, W)
    return run_bass_kernel_spmd(nc, in_maps, list(range(NCORES)), **spmd_kwargs)


def kernel(x, W):
    res = _run(x, W)
    return _assemble(res.results)
